# revision 68
# baseline (speedup 1.0000x reference)
"""Kimi-style MoE (8 routed experts top-2 + shared expert) on 8 Trainium2 cores.

Sharding: expert-parallel with TRUE sparse routing. Core c owns routed expert c
and a 1/8 intermediate-dim shard of the shared expert. The fp32 gate is
replicated; each core then *compacts* the ids of the tokens routed to its
expert fully on-device (triangular-matmul prefix sum -> one-hot slot matrix ->
payload matmul extracting ids), gathers just those token rows with an indirect
DMA, runs the expert MLP on C=768 token slots instead of all 2048, and returns
the compacted (unweighted) rows + token ids + gate weights. The host applies
the combine weights and scatter-adds the rows into the dense shared partials.

Schedule: the dense shared-expert up-projection is interleaved with the gate
stream and the routing pipeline so the PE never starves; weights/consts load
on the Activation HWDGE queue while the token stream uses SP; psum->sbuf
copies for the transposes run on gpsimd.
"""

import sys

for _p in ("/opt/trn_rl_repo", "/opt/pypackages"):
    if _p not in sys.path:
        sys.path.insert(0, _p)

import numpy as np
import ml_dtypes

import concourse.bass as bass
import concourse.mybir as mybir
import concourse.tile as tile
from concourse import bacc
from concourse.bass import ts, IndirectOffsetOnAxis
from concourse.bass_utils import run_bass_kernel_spmd
from concourse.masks import make_identity

BF16 = mybir.dt.bfloat16
F32 = mybir.dt.float32
I32 = mybir.dt.int32
NP_BF16 = ml_dtypes.bfloat16

# Problem shapes (hardcoded per the contract).
B, S, D = 2, 1024, 1024
E, TOPK = 8, 2
I = 1408
N_SHARED = 2
I_SH = N_SHARED * I          # 2816
SCALE = 2.5
T = B * S                    # 2048
P = 128
MT = T // P                  # 16 token tiles (gate)
KO = D // P                  # 8 contraction subtiles over D
JR = I // P                  # 11 routed (v,g) pair tiles
JS = 3                       # shared pair tiles per core (padded)
KD = JR + JS                 # 14 down-proj contraction tiles
DT = D // P                  # 8 output partition tiles
N_CORES = 8

C = 768                      # routed token capacity per expert (6 x 128)
NG = C // P                  # 6 gather tiles
CF = [(0, 512), (512, 256)]  # free-dim tiling of the C token slots
TF = 512                     # shared-expert free tile
NT = T // TF                 # 4
BIG = 1.0e9
OOB = 6000.0                 # tail sentinel (> T-1 -> indirect DMA skips)

# packed fp32 const columns
GW0, GB0, SEL0, B10, BS10, B20, BS20, IO0 = 0, 64, 72, 80, 102, 108, 116, 124
NF32 = IO0 + C               # 892
# packed bf16 const columns
TRI0, PAY0 = 0, 128
NBF = PAY0 + 3 * MT          # 176


def _body(tc, io, uid=0):
    nc = tc.nc
    add = mybir.AluOpType.add
    mult = mybir.AluOpType.mult
    sub = mybir.AluOpType.subtract
    SIG = mybir.ActivationFunctionType.Sigmoid

    with (
        tc.tile_pool(name="const", bufs=1) as cpool,
        tc.tile_pool(name="sv", bufs=4) as svpool,
        tc.tile_pool(name="wexp", bufs=1) as bpool,
    ):
        # ---- resident SBUF tensors ----
        w1a = bpool.tile([P, 6, KO, 2 * P], BF16, tag="w1a")
        w1b = bpool.tile([P, JR - 6, KO, 2 * P], BF16, tag="w1b")
        ws1s = []
        for jj in range(JS):
            ws1_jj = bpool.tile([P, KO, 2 * P], BF16, tag=f"ws1_{jj}", name=f"ws1_{jj}")
            ws1s.append(ws1_jj)
        cf = cpool.tile([P, NF32], F32, tag="cf")
        cb = cpool.tile([P, NBF], BF16, tag="cb")
        gb = cpool.tile([P, E], F32, tag="gb")
        sel = cpool.tile([P, E], F32, tag="sel")
        ident16 = cpool.tile([P, P], BF16, tag="ident16")
        ones1 = cpool.tile([1, P], F32, tag="ones1")
        h_s = cpool.tile([P, JS, T], BF16, tag="h_s")         # shared swiglu out
        xTg = bpool.tile([P, KO, C], BF16, tag="xTg")         # gathered x, [d, tok]

        # token stream on SP queue; small consts + shared weights on the
        # Activation queue (the big routed-weight DMAs are emitted later so
        # they don't block the early sigmoids / gate stream)
        nc.scalar.dma_start(cf[:], io["cf32"][:])
        nc.scalar.dma_start(cb[:], io["cbf"][:])
        make_identity(nc, ident16[:])
        nc.vector.memset(ones1[:], 1.0)
        nc.vector.tensor_copy(gb[:], cf[:, GB0 : GB0 + E])
        nc.vector.tensor_copy(sel[:], cf[:, SEL0 : SEL0 + E])

        s_all = cpool.tile([P, MT, E], F32, tag="s_all")

        def sup_block(jj, t):
            """one shared-expert up-proj block: [128, TF] swiglu -> h_s."""
            pv = pvpool.tile([P, TF], F32, tag="pv")
            pgu = pgupool.tile([P, TF], F32, tag="pgu")
            for k in range(KO):
                nc.tensor.matmul(
                    pv[:], ws1s[jj][:, k, :P], xT16[:, k, ts(t, TF)],
                    start=(k == 0), stop=(k == KO - 1),
                )
            for k in range(KO):
                nc.tensor.matmul(
                    pgu[:], ws1s[jj][:, k, P:], xT16[:, k, ts(t, TF)],
                    start=(k == 0), stop=(k == KO - 1),
                )
            sv = svpool.tile([P, TF], F32, tag="sv")
            bias_v = cf[:, BS10 + 2 * jj : BS10 + 2 * jj + 1]
            bias_g = cf[:, BS10 + 2 * jj + 1 : BS10 + 2 * jj + 2]
            nc.scalar.activation(sv[:], pv[:], SIG, bias=bias_v)
            nc.vector.scalar_tensor_tensor(sv[:], pv[:], bias_v, sv[:], add, mult)
            nc.vector.scalar_tensor_tensor(
                h_s[:, jj, ts(t, TF)], pgu[:], bias_g, sv[:], add, mult
            )

        with tc.tile_pool(name="pvp", bufs=2, space="PSUM") as pvpool, \
             tc.tile_pool(name="pgp", bufs=1, space="PSUM") as pgupool:
            xpool_cm = tc.tile_pool(name="xbig", bufs=1)
            xpool = xpool_cm.__enter__()
            xT16 = xpool.tile([P, KO, T], BF16, tag="xT16")
            xg_all = xpool.tile([P, NG, D], BF16, tag="xg_all")
            nc.vector.memset(xg_all[:], 0)

            # ---- compaction state ----
            gtmp = cpool.tile([P, 4, E], F32, tag="gtmp")
            gtmp2 = cpool.tile([P, 4, E], F32, tag="gtmp2")
            m1 = cpool.tile([P, 4], F32, tag="m1")
            m2 = cpool.tile([P, 4], F32, tag="m2")
            wq = cpool.tile([P, MT], F32, tag="wq")
            mask32 = cpool.tile([P, MT], F32, tag="mask32")
            mask16 = cpool.tile([P, MT], BF16, tag="mask16")
            cum32 = cpool.tile([P, MT], F32, tag="cum32")
            offs32 = cpool.tile([P, 4], F32, tag="offs32")
            posm = cpool.tile([P, MT], F32, tag="posm")
            sa = cpool.tile([1, 4], F32, tag="sa")
            sb = cpool.tile([1, 4], F32, tag="sb")
            sc = cpool.tile([1, 4], F32, tag="sc")
            sd = cpool.tile([1, 4], F32, tag="sd")
            base = cpool.tile([1, 1], F32, tag="base")
            ex = cpool.tile([3, C], F32, tag="ex")
            idfp = cpool.tile([P, NG], F32, tag="idfp")
            ones128 = cpool.tile([P, 1], BF16, tag="ones128")
            ident32 = cpool.tile([P, P], F32, tag="ident32")
            idx32 = cpool.tile([P, NG], I32, tag="idx32")
            nc.vector.memset(ones128[:], 1.0)
            nc.vector.memset(base[:], 0.0)
            make_identity(nc, ident32[:])

            XA = mybir.AxisListType.X
            iseq = mybir.AluOpType.is_equal

            def topk_group(g):
                """exact fp32 top-2 for token tiles 4g..4g+3 -> wq/mask."""
                sl = slice(4 * g, 4 * g + 4)
                s_g = s_all[:, sl]
                nc.vector.tensor_tensor(
                    s_g, s_g, gb[:, None, :].to_broadcast((P, 4, E)), add
                )
                nc.vector.reduce_max(m1[:], s_g, axis=XA)
                nc.vector.tensor_tensor(
                    gtmp[:], s_g, m1[:, :, None].to_broadcast((P, 4, E)), iseq
                )
                nc.vector.scalar_tensor_tensor(gtmp2[:], gtmp[:], -BIG, s_g, mult, add)
                nc.vector.reduce_max(m2[:], gtmp2[:], axis=XA)
                nc.vector.tensor_tensor(
                    gtmp2[:], gtmp2[:], m2[:, :, None].to_broadcast((P, 4, E)), iseq
                )
                nc.vector.tensor_tensor(gtmp[:], gtmp[:], gtmp2[:], add)
                nc.vector.tensor_tensor(gtmp[:], gtmp[:], s_g, mult)
                nc.vector.tensor_tensor(
                    gtmp[:], gtmp[:], sel[:, None, :].to_broadcast((P, 4, E)), mult
                )
                nc.vector.reduce_sum(wq[:, sl], gtmp[:], axis=XA)
                nc.vector.tensor_tensor(m1[:], m1[:], m2[:], add)
                nc.vector.reciprocal(m2[:], m1[:])
                nc.vector.tensor_scalar_mul(m2[:], m2[:], SCALE)
                nc.vector.tensor_tensor(wq[:, sl], wq[:, sl], m2[:], mult)
                nc.vector.tensor_scalar(
                    mask32[:, sl], wq[:, sl], 0.0, None, mybir.AluOpType.is_gt
                )
                nc.vector.tensor_copy(mask16[:, sl], mask32[:, sl])

            def compact_group(g, ring, ex0, ex1, ohpool):
                """slot positions for tiles 4g..4g+3 + one-hot payload matmuls."""
                sl = slice(4 * g, 4 * g + 4)
                rt = ring.tile([P, E], F32, tag="rt")
                for i in range(4):
                    mt = 4 * g + i
                    nc.tensor.matmul(
                        rt[:, i : i + 1], cb[:, TRI0 : TRI0 + P],
                        mask16[:, mt : mt + 1], start=True, stop=True,
                    )
                nc.tensor.matmul(
                    rt[0:1, 4:8], ones128[:], mask16[:, sl], start=True, stop=True
                )
                nc.vector.tensor_copy(cum32[:, sl], rt[:, 0:4])
                nc.vector.tensor_copy(sa[:], rt[0:1, 4:8])
                nc.vector.tensor_copy(sb[:], sa[:])
                nc.vector.tensor_tensor(sb[0:1, 1:4], sa[0:1, 1:4], sa[0:1, 0:3], add)
                nc.vector.tensor_copy(sc[:], sb[:])
                nc.vector.tensor_tensor(sc[0:1, 2:4], sb[0:1, 2:4], sb[0:1, 0:2], add)
                nc.vector.tensor_tensor(sd[:], sc[:], sa[:], sub)
                nc.vector.tensor_tensor(
                    sd[:], sd[:], base[0:1, 0:1].to_broadcast((1, 4)), add
                )
                nc.vector.tensor_tensor(base[:], base[:], sc[0:1, 3:4], add)
                ot = ring.tile([P, E], F32, tag="rt")
                nc.tensor.matmul(ot[:, 0:4], ones1[:], sd[:], start=True, stop=True)
                nc.vector.tensor_copy(offs32[:], ot[:, 0:4])
                nc.vector.tensor_tensor(posm[:, sl], cum32[:, sl], offs32[:], add)
                nc.vector.scalar_tensor_tensor(
                    posm[:, sl], posm[:, sl], -(1.0 + OOB), mask32[:, sl], add, mult
                )
                nc.vector.tensor_scalar_add(posm[:, sl], posm[:, sl], OOB)
                for i in range(4):
                    mt = 4 * g + i
                    oh = ohpool.tile([P, C], BF16, tag="oh")
                    nc.vector.tensor_scalar(
                        oh[:], cf[:, IO0 : IO0 + C], posm[:, mt : mt + 1], None, iseq
                    )
                    nc.tensor.matmul(
                        ex0[:], cb[:, PAY0 + 3 * mt : PAY0 + 3 * mt + 3],
                        oh[:, 0:512], start=(mt == 0), stop=(mt == MT - 1),
                    )
                    nc.tensor.matmul(
                        ex1[:], cb[:, PAY0 + 3 * mt : PAY0 + 3 * mt + 3],
                        oh[:, 512:C], start=(mt == 0), stop=(mt == MT - 1),
                    )

            # ---- gate (fp32) + pipelined compaction + shared-up fillers ----
            ring_cm = tc.tile_pool(name="ring", bufs=3, space="PSUM")
            ring = ring_cm.__enter__()
            eps_cm = tc.tile_pool(name="epsum", bufs=1, space="PSUM")
            eps = eps_cm.__enter__()
            oh_cm = tc.tile_pool(name="oh", bufs=3)
            ohpool = oh_cm.__enter__()
            ex0 = eps.tile([3, 512], F32, tag="ex0")
            ex1 = eps.tile([3, 256], F32, tag="ex1")
            with tc.tile_pool(name="gx", bufs=4) as gxpool:
                sup_sched = {5: (0, 0), 7: (0, 1), 9: (1, 0), 11: (0, 2), 13: (1, 1)}
                for mt in range(MT):
                    if mt in (3, 7, 11):
                        jj = (mt - 3) // 4
                        nc.sync.dma_start(ws1s[jj][:], io["ws1t"][:, jj])
                    xg32 = gxpool.tile([P, KO, P], F32, tag="xg32")
                    nc.sync.dma_start(xg32[:], io["xT32"][mt])
                    pg = ring.tile([P, E], F32, tag="rt")
                    for k in range(KO):
                        nc.tensor.matmul(
                            pg[:], xg32[:, k], cf[:, GW0 + 8 * k : GW0 + 8 * k + 8],
                            start=(k == 0), stop=(k == KO - 1),
                        )
                    nc.scalar.activation(s_all[:, mt], pg[:], SIG)
                    # bf16 transposed activations for the shared expert
                    nc.gpsimd.tensor_copy(xT16[:, :, ts(mt, P)], xg32[:])
                    if mt in sup_sched:
                        sup_block(*sup_sched[mt])
                    if mt % 4 == 3:
                        topk_group(mt // 4)
                        compact_group(mt // 4, ring, ex0, ex1, ohpool)

            nc.sync.dma_start(io["out_wq"][:], wq[:])

            # ---- ids: transpose payload rows onto slot partitions ----
            nc.vector.tensor_copy(ex[:, 0:512], ex0[:])
            nc.vector.tensor_copy(ex[:, 512:C], ex1[:])
            tps = cpool.tile([P, 3 * NG], F32, tag="tps")
            for g in range(NG):
                tp = ring.tile([P, E], F32, tag="rt")
                nc.tensor.transpose(
                    tp[:, 0:3], ex[0:3, ts(g, P)], ident32[0:3, 0:3]
                )
                nc.vector.tensor_copy(tps[:, 3 * g : 3 * g + 3], tp[:, 0:3])
                nc.vector.scalar_tensor_tensor(
                    idfp[:, g : g + 1], tps[:, 3 * g + 1 : 3 * g + 2], 128.0,
                    tps[:, 3 * g : 3 * g + 1], mult, add,
                )
                nc.vector.scalar_tensor_tensor(
                    idfp[:, g : g + 1], tps[:, 3 * g + 2 : 3 * g + 3], -OOB,
                    idfp[:, g : g + 1], mult, add,
                )
            nc.vector.tensor_scalar_add(idfp[:], idfp[:], OOB)
            nc.vector.tensor_copy(idx32[:], idfp[:])
            nc.sync.dma_start(io["out_ids"][:], idfp[:])

            oh_cm.__exit__(None, None, None)
            eps_cm.__exit__(None, None, None)
            ring_cm.__exit__(None, None, None)

            # ---- indirect gather of this expert's token rows (gpsimd) ----
            # the gpsimd queue sequences the DMA bus: w1a fills the idx-wait
            # gap, w1b the transpose-wait gap, wd rides behind the copies
            nc.gpsimd.dma_start(w1a[:], io["w1t"][:, 0:6])
            for g in range(NG):
                nc.gpsimd.indirect_dma_start(
                    out=xg_all[:, g],
                    out_offset=None,
                    in_=io["x16r"][:],
                    in_offset=IndirectOffsetOnAxis(ap=idx32[:, g : g + 1], axis=0),
                    bounds_check=T - 1,
                    oob_is_err=False,
                )
            for (jj, t) in ((1, 2), (0, 3), (1, 3)):
                sup_block(jj, t)
            for t in range(NT):
                sup_block(2, t)

            # ---- transpose gathered rows -> xTg [d, slot] ----
            with tc.tile_pool(name="xtp", bufs=4, space="PSUM") as xtp:
                for g in range(NG):
                    for k in range(KO):
                        tpx = xtp.tile([P, P], BF16, tag="tpx")
                        nc.tensor.transpose(
                            tpx[:], xg_all[:, g, ts(k, P)], ident16[:]
                        )
                        if (g * KO + k) % 2:
                            nc.scalar.activation(
                                xTg[:, k, ts(g, P)], tpx[:],
                                mybir.ActivationFunctionType.Copy,
                            )
                        else:
                            nc.vector.tensor_copy(xTg[:, k, ts(g, P)], tpx[:])
            nc.gpsimd.dma_start(w1b[:], io["w1t"][:, 6:JR])

            # xT16/xg_all are dead from here; free them before the late pools
            xpool_cm.__exit__(None, None, None)

            # down-proj weights arrive while the routed up-projection runs
            late_pool = tc.tile_pool(name="late", bufs=1)
            late = late_pool.__enter__()
            wd = late.tile([P, KD, DT, P], BF16, tag="wd")
            h_r = late.tile([P, JR, C], BF16, tag="h_r")
            nc.gpsimd.dma_start(wd[:], io["wd"][:])

            # ---- routed expert up (sparse, C slots) ----
            for j in range(JR):
                w1j = w1a[:, j] if j < 6 else w1b[:, j - 6]
                for (f0, fw) in CF:
                    pv = pvpool.tile([P, fw], F32, tag="pv")
                    pgu = pgupool.tile([P, fw], F32, tag="pgu")
                    for k in range(KO):
                        nc.tensor.matmul(
                            pv[:], w1j[:, k, :P], xTg[:, k, f0 : f0 + fw],
                            start=(k == 0), stop=(k == KO - 1),
                        )
                    for k in range(KO):
                        nc.tensor.matmul(
                            pgu[:], w1j[:, k, P:], xTg[:, k, f0 : f0 + fw],
                            start=(k == 0), stop=(k == KO - 1),
                        )
                    sv = svpool.tile([P, fw], F32, tag="sv")
                    bias_v = cf[:, B10 + 2 * j : B10 + 2 * j + 1]
                    bias_g = cf[:, B10 + 2 * j + 1 : B10 + 2 * j + 2]
                    nc.scalar.activation(sv[:], pv[:], SIG, bias=bias_v)
                    nc.vector.scalar_tensor_tensor(
                        sv[:], pv[:], bias_v, sv[:], add, mult
                    )
                    nc.vector.scalar_tensor_tensor(
                        h_r[:, j, f0 : f0 + fw], pgu[:], bias_g, sv[:], add, mult
                    )

        # ---- routed down (+b2, unweighted) -> [d, slot] ; shared down -> [D,T] ----
        with (
            tc.tile_pool(name="dpsum", bufs=2, space="PSUM") as dpsum,
            tc.tile_pool(name="outs", bufs=4) as opool,
            tc.tile_pool(name="orf", bufs=2) as orfpool,
        ):
            for fi, (f0, fw) in enumerate(CF):
                or_f = orfpool.tile([P, DT, fw], BF16, tag="or_f")
                for dt in range(DT):
                    pd = dpsum.tile([P, fw], F32, tag="pd")
                    for kd in range(JR):
                        nc.tensor.matmul(
                            pd[:], wd[:, kd, dt], h_r[:, kd, f0 : f0 + fw],
                            start=(kd == 0), stop=(kd == JR - 1),
                        )
                    nc.vector.tensor_scalar(
                        or_f[:, dt], pd[:], cf[:, B20 + dt : B20 + dt + 1], None, add
                    )
                # natural [d, slot] orientation; the host transposes in combine
                nc.sync.dma_start(io["out_r"][:, :, f0 : f0 + fw], or_f[:])

            for dt in range(DT):
                for t in range(NT):
                    pds = dpsum.tile([P, TF], F32, tag="pd")
                    for jj in range(JS):
                        nc.tensor.matmul(
                            pds[:], wd[:, JR + jj, dt], h_s[:, jj, ts(t, TF)],
                            start=(jj == 0), stop=(jj == JS - 1),
                        )
                    osb = opool.tile([P, TF], BF16, tag="osb")
                    nc.scalar.activation(
                        osb[:], pds[:], mybir.ActivationFunctionType.Identity,
                        bias=cf[:, BS20 + dt : BS20 + dt + 1],
                    )
                    nc.sync.dma_start(io["out_sh"][ts(dt, P), ts(t, TF)], osb[:])

        late_pool.__exit__(None, None, None)


def build_nc(reps=1):
    nc = bacc.Bacc(None, target_bir_lowering=False, debug=False)
    io = {
        "xT32": nc.declare_dram_parameter("xT32", [MT, P, KO, P], F32, isOutput=False),
        "x16r": nc.declare_dram_parameter("x16r", [T, D], BF16, isOutput=False),
        "cf32": nc.declare_dram_parameter("cf32", [P, NF32], F32, isOutput=False),
        "cbf": nc.declare_dram_parameter("cbf", [P, NBF], BF16, isOutput=False),
        "w1t": nc.declare_dram_parameter(
            "w1t", [P, JR, KO, 2 * P], BF16, isOutput=False
        ),
        "ws1t": nc.declare_dram_parameter(
            "ws1t", [P, JS, KO, 2 * P], BF16, isOutput=False
        ),
        "wd": nc.declare_dram_parameter("wd", [P, KD, DT, P], BF16, isOutput=False),
        "out_sh": nc.declare_dram_parameter("out_sh", [D, T], BF16, isOutput=True),
        "out_r": nc.declare_dram_parameter("out_r", [P, DT, C], BF16, isOutput=True),
        "out_ids": nc.declare_dram_parameter("out_ids", [P, NG], F32, isOutput=True),
        "out_wq": nc.declare_dram_parameter("out_wq", [P, MT], F32, isOutput=True),
    }
    with tile.TileContext(nc) as tc:
        for r in range(reps):
            _body(tc, io, uid=r)
    nc.compile()
    return nc


def _part_tiles(vec, n_tiles):
    """[n_tiles*128] -> [128, n_tiles] (partition-tiled per-row constants)."""
    return np.ascontiguousarray(vec.reshape(n_tiles, P).T.astype(np.float32))


def _shared_slices(core):
    """Global shared pair-tile indices owned by `core` (<= JS of them)."""
    counts = [3, 3, 3, 3, 3, 3, 2, 2]
    start = sum(counts[:core])
    return list(range(start, start + counts[core]))


def prep_inputs(inputs):
    """Full problem inputs -> list of 8 per-core in_maps (numpy arrays)."""
    x = np.asarray(inputs["x"], np.float32)
    gate_w = np.asarray(inputs["gate_w"], np.float32)
    gate_bias = np.asarray(inputs["gate_bias"], np.float32)
    W1 = np.asarray(inputs["W1"], np.float32)
    b1 = np.asarray(inputs["b1"], np.float32)
    W2 = np.asarray(inputs["W2"], np.float32)
    b2 = np.asarray(inputs["b2"], np.float32)
    Ws1 = np.asarray(inputs["Ws1"], np.float32)
    bs1 = np.asarray(inputs["bs1"], np.float32)
    Ws2 = np.asarray(inputs["Ws2"], np.float32)
    bs2 = np.asarray(inputs["bs2"], np.float32)

    xf = x.reshape(T, D)
    # tile-major: xT32[mt, p, ko, pp] = xf[mt*128+pp, ko*128+p] (4KB runs/partition)
    xT32 = np.ascontiguousarray(
        xf.T.reshape(KO, P, MT, P).transpose(2, 1, 0, 3)
    )
    x16r = np.ascontiguousarray(xf).astype(NP_BF16)
    gwT = np.ascontiguousarray(gate_w.T.reshape(KO, P, E).transpose(1, 0, 2))

    cbf = np.zeros((P, NBF), np.float32)
    cbf[:, TRI0 : TRI0 + P] = np.triu(np.ones((P, P), np.float32))
    pay = np.zeros((P, MT, 3), np.float32)
    pay[:, :, 0] = np.arange(P)[:, None]
    pay[:, :, 1] = np.arange(MT)[None, :]
    pay[:, :, 2] = 1.0
    cbf[:, PAY0:] = pay.reshape(P, 3 * MT)
    cbf = cbf.astype(NP_BF16)

    in_maps = []
    for c in range(N_CORES):
        A = W1[c].reshape(2, JR, P, KO, P)  # (vg, j, m, ko, p)
        w1t = np.ascontiguousarray(
            A.transpose(4, 1, 3, 0, 2).reshape(P, JR, KO, 2 * P)
        ).astype(NP_BF16)
        b1t = np.ascontiguousarray(
            b1[c].reshape(2, JR, P).transpose(2, 1, 0).reshape(P, 2 * JR)
        )

        sl = _shared_slices(c)
        A_sh = np.zeros((2, JS, P, D), np.float32)
        bs1t_raw = np.zeros((2, JS, P), np.float32)
        Wd_sh = np.zeros((JS, P, D), np.float32)
        for jj, jglob in enumerate(sl):
            rows = slice(jglob * P, (jglob + 1) * P)
            A_sh[0, jj] = Ws1[rows.start : rows.stop]
            A_sh[1, jj] = Ws1[I_SH + rows.start : I_SH + rows.stop]
            bs1t_raw[0, jj] = bs1[rows]
            bs1t_raw[1, jj] = bs1[I_SH + rows.start : I_SH + rows.stop]
            Wd_sh[jj] = Ws2[:, rows].T
        ws1t = np.ascontiguousarray(
            A_sh.reshape(2, JS, P, KO, P).transpose(4, 1, 3, 0, 2).reshape(
                P, JS, KO, 2 * P
            )
        ).astype(NP_BF16)
        bs1t = np.ascontiguousarray(bs1t_raw.transpose(2, 1, 0).reshape(P, 2 * JS))

        Wd = np.concatenate([W2[c].T, Wd_sh.reshape(JS * P, D)], axis=0)
        wd = np.ascontiguousarray(
            Wd.reshape(KD, P, DT, P).transpose(1, 0, 2, 3)
        ).astype(NP_BF16)

        sel_b = np.zeros((P, E), np.float32)
        sel_b[:, c] = 1.0
        bs2_c = bs2 if c == 0 else np.zeros_like(bs2)

        cf32 = np.zeros((P, NF32), np.float32)
        cf32[:, GW0 : GW0 + 64] = gwT.reshape(P, KO * E)
        cf32[:, GB0 : GB0 + E] = gate_bias[None, :]
        cf32[:, SEL0 : SEL0 + E] = sel_b
        cf32[:, B10 : B10 + 2 * JR] = b1t
        cf32[:, BS10 : BS10 + 2 * JS] = bs1t
        cf32[:, B20 : B20 + DT] = _part_tiles(b2[c], DT)
        cf32[:, BS20 : BS20 + DT] = _part_tiles(bs2_c, DT)
        cf32[:, IO0 : IO0 + C] = np.arange(C, dtype=np.float32)[None, :]

        in_maps.append(
            {
                "xT32": xT32,
                "x16r": x16r,
                "cf32": cf32,
                "cbf": cbf,
                "w1t": w1t,
                "ws1t": ws1t,
                "wd": wd,
            }
        )
    return in_maps


_NC_CACHE = {}


def get_nc():
    if "nc" not in _NC_CACHE:
        _NC_CACHE["nc"] = build_nc()
    return _NC_CACHE["nc"]


def combine_outputs(results):
    """Per-core result dicts -> full [B, S, D] float32 output."""
    acc = np.zeros((T, D), np.float32)
    for r in results:
        acc += np.asarray(r["out_sh"], np.float32).T
        ids = (
            np.ascontiguousarray(np.asarray(r["out_ids"], np.float32).T)
            .ravel()
            .astype(np.int64)
        )
        rows = np.ascontiguousarray(
            np.asarray(r["out_r"], np.float32).transpose(2, 1, 0)
        ).reshape(C, D)
        wqf = np.ascontiguousarray(np.asarray(r["out_wq"], np.float32).T).ravel()
        valid = ids < T
        iv = ids[valid]
        acc[iv] += rows[valid] * wqf[iv][:, None]
    return np.ascontiguousarray(acc.reshape(B, S, D))


def kernel(**inputs):
    nc = get_nc()
    in_maps = prep_inputs(inputs)
    res = run_bass_kernel_spmd(nc, in_maps, core_ids=list(range(N_CORES)))
    return combine_outputs(res.results)


if __name__ == "__main__":
    # quick self-drive (requires reference.py next to this file)
    import reference

    inputs = {k: np.asarray(v) for k, v in reference.setup_inputs().items()}
    out = kernel(**inputs)
    exp = np.asarray(reference.reference(**inputs))
    err = np.abs(out - exp).max()
    rel = np.abs(out - exp).max() / np.abs(exp).max()
    print("absmax err:", err, "rel:", rel)
